# revision 1
# baseline (speedup 1.0000x reference)
"""AdaGNN on 8 TRN2 NeuronCores (Bass, SPMD).

Strategy (node sharding, replicated graph tables):
- Host packs the 50000 nodes into 8 cores x 49 blocks x 128 rows via a
  load-balancing permutation (block in-degree sums <= TPB*128).
- spmm per 128-slot tile: one K=1 indirect DMA gathers the 128 source rows
  (one per partition) from the DRAM feature table; the DVE builds a
  (128 slots x 128 rows) one-hot-times-val matrix; the PE accumulates
  psum_block += S_vals.T @ gathered.  Graph structure (gather indices,
  dest-locals, vals) is identical for all 4 spmm layers.
- Dense layers run on-chip in bf16 (PE transposes + matmuls); h is
  broadcast between layers with an AllGather collective; log_softmax is
  computed on-chip; the host un-permutes the final (50000, 40) output.
"""

import math
import numpy as np
import ml_dtypes

import concourse.bacc as bacc
import concourse.bass as bass
import concourse.mybir as mybir
from concourse.bass_utils import run_bass_kernel_spmd

N = 50000
E = 800000
NFEAT = 256
NHID = 128
NCLASS = 40
CORES = 8
P = 128
BPC = 49                 # blocks per core
NBLK = CORES * BPC       # 392
RPC = BPC * P            # 6272 rows per core
NPACK = NBLK * P         # 50176 packed rows
NBUF = 32                # gather buffer ring
NSV = 32                 # svals ring

F32 = mybir.dt.float32
BF16 = mybir.dt.bfloat16
I32 = mybir.dt.int32
BF = ml_dtypes.bfloat16


# ----------------------------------------------------------------------------
# Host-side graph packing
# ----------------------------------------------------------------------------

def _pack_graph(rows, cols, vals):
    """Assign nodes to 392 blocks of 128 balancing in-degree sums; build
    per-core per-tile gather indices / dest-locals / vals."""
    indeg = np.bincount(rows, minlength=N).astype(np.int64)
    order = np.argsort(-indeg, kind="stable")
    import heapq
    heap = [(0, b) for b in range(NBLK)]
    heapq.heapify(heap)
    bcount = np.zeros(NBLK, np.int64)
    bsum = np.zeros(NBLK, np.int64)
    pos = np.empty(N, np.int64)
    for v in order:
        while True:
            s, b = heapq.heappop(heap)
            if bcount[b] < P:
                break
        pos[v] = b * P + bcount[b]
        bcount[b] += 1
        bsum[b] += indeg[v]
        if bcount[b] < P:
            heapq.heappush(heap, (bsum[b], b))
    tpb = int(math.ceil(bsum.max() / P))
    # Table row order: region A = all cores' blocks 0..39, region B = rest, so
    # each AllGather half writes one contiguous table region and the first half
    # can fire mid-layer, overlapped with the tail of the gather stream.
    SPLIT = 40
    blk_all = pos // P
    loc_all = pos % P
    c_all = blk_all // BPC
    b_all = blk_all % BPC
    tpos = np.where(
        b_all < SPLIT,
        (SPLIT * P) * c_all + P * b_all + loc_all,
        CORES * SPLIT * P + (BPC - SPLIT) * P * c_all + P * (b_all - SPLIT) + loc_all,
    )
    d_pos = pos[rows]
    s_pos = tpos[cols]
    blk = d_pos // P
    dloc = d_pos % P
    eorder = np.argsort(blk, kind="stable")
    blk_s = blk[eorder]
    dloc_s = (dloc[eorder]).astype(np.float32)
    src_s = s_pos[eorder].astype(np.int32)
    val_s = np.asarray(vals)[eorder].astype(np.float32)
    starts = np.searchsorted(blk_s, np.arange(NBLK + 1))
    T = BPC * tpb
    gidx = np.zeros((CORES, P, T), np.int32)
    gdl = np.zeros((CORES, P, T), np.float32)
    gvl = np.zeros((CORES, P, T), np.float32)
    cap = tpb * P
    A_ROWS = CORES * SPLIT * P
    nA0 = np.full(CORES, 10**9, np.int64)
    for b in range(NBLK):
        lo, hi = starts[b], starts[b + 1]
        n = hi - lo
        assert n <= cap, f"block {b} has {n} edges > cap {cap}"
        c, bc = b // BPC, b % BPC
        # region-A sources first, so leading tiles can gather before the
        # boundary collective (which only writes region B) completes
        ro = np.argsort(src_s[lo:hi] >= A_ROWS, kind="stable")
        sA = src_s[lo:hi][ro]
        dA = dloc_s[lo:hi][ro]
        vA = val_s[lo:hi][ro]
        if bc == 0:
            nA0[c] = int((sA < A_ROWS).sum())
        sl = np.arange(n)
        tt = bc * tpb + sl // P
        pp = sl % P
        gidx[c, pp, tt] = sA
        gdl[c, pp, tt] = dA
        gvl[c, pp, tt] = vA
    W = int(min(13, nA0.min() // P))
    return pos, tpos, tpb, W, gidx, gdl, gvl


# ----------------------------------------------------------------------------
# Bass graph
# ----------------------------------------------------------------------------

def _build(tpb, W):
    T = BPC * tpb
    SPLIT = 40
    FDIMS = [NFEAT, NHID, NHID, NHID]
    NINIT = 13 + BPC

    nc = bacc.Bacc("TRN2")

    xt = nc.declare_dram_parameter("xt", [NPACK, NFEAT], BF16, isOutput=False)
    xloc = nc.declare_dram_parameter("xloc", [RPC, NFEAT], BF16, isOutput=False)
    gidx = nc.declare_dram_parameter("gidx", [P, T], I32, isOutput=False)
    gdl = nc.declare_dram_parameter("gdl", [P, T], F32, isOutput=False)
    gvl = nc.declare_dram_parameter("gvl", [P, T], F32, isOutput=False)
    w1p = nc.declare_dram_parameter("w1p", [P, 2 * NHID], BF16, isOutput=False)
    w2p = nc.declare_dram_parameter("w2p", [P, NCLASS], BF16, isOutput=False)
    b1p = nc.declare_dram_parameter("b1p", [P, 1], F32, isOutput=False)
    b2p = nc.declare_dram_parameter("b2p", [NCLASS, 1], F32, isOutput=False)
    d1p = nc.declare_dram_parameter("d1p", [P, NFEAT], F32, isOutput=False)
    hdp = nc.declare_dram_parameter("hdp", [P, 2 * NHID], F32, isOutput=False)
    d2p = nc.declare_dram_parameter("d2p", [P, NHID], F32, isOutput=False)
    iop = nc.declare_dram_parameter("iop", [P, P], F32, isOutput=False)
    idbf = nc.declare_dram_parameter("idbf", [P, P], BF16, isOutput=False)
    idf32 = nc.declare_dram_parameter("idf32", [P, P], F32, isOutput=False)
    outp = nc.declare_dram_parameter("out", [RPC, NCLASS], F32, isOutput=True)

    h_shard = nc.dram_tensor("h_shard", [RPC, NHID], BF16)
    h_tA = nc.dram_tensor("h_tA", [NPACK, NHID], BF16)
    h_tB = nc.dram_tensor("h_tB", [NPACK, NHID], BF16)

    import contextlib
    ctx = contextlib.ExitStack()
    block = ctx.enter_context(nc.Block())
    sem = {}
    for nm in ["init", "dveinit", "cc", "hw", "os"]:
        sem[nm] = ctx.enter_context(nc.semaphore(nm))
    for L in range(4):
        for nm in ["g", "s", "mm", "t", "u1", "v1", "u2", "v2", "u3", "v3",
                   "u4", "m1", "a2", "m2"]:
            sem[f"{nm}{L}"] = ctx.enter_context(nc.semaphore(f"{nm}{L}"))

    sb = {}
    def S(name, shape, dt):
        sb[name] = ctx.enter_context(nc.sbuf_tensor(name, shape, dt))
        return sb[name]

    gbuf = S("gbuf", [P, NBUF, NFEAT], BF16)
    sval = S("sval", [P, NSV, P], BF16)
    idxs = S("idxs", [P, T], I32)
    dls = S("dls", [P, T], F32)
    vls = S("vls", [P, T], F32)
    w1s = S("w1s", [P, 2 * NHID], BF16)
    w2s = S("w2s", [P, NCLASS], BF16)
    b1s = S("b1s", [P, 1], F32)
    b2s = S("b2s", [NCLASS, 1], F32)
    d1s = S("d1s", [P, NFEAT], F32)
    hds = S("hds", [P, 2 * NHID], F32)
    d2s = S("d2s", [P, NHID], F32)
    ios = S("ios", [P, P], F32)
    idb = S("idb", [P, P], BF16)
    idf = S("idf", [P, P], F32)
    zer = S("zer", [P, NFEAT], BF16)
    xow = S("xow", [P, BPC, NFEAT], BF16)
    how = S("how", [P, BPC, NHID], BF16)
    tmp = S("tmp", [P, NFEAT], BF16)
    hpre = S("hpre", [P, NFEAT], BF16)
    hpT = S("hpT", [P, 2 * NHID], BF16)
    hTs = S("hTs", [P, NHID], BF16)
    p4T = S("p4T", [P, P], BF16)
    oTs = S("oTs", [NCLASS, P], F32)
    nmx = S("nmx", [P, 1], F32)
    sxp = S("sxp", [P, 1], F32)
    lse = S("lse", [P, 1], F32)
    esb = S("esb", [P, NCLASS], F32)
    osb = S("osb", [P, 2, NCLASS], F32)

    ps = {}
    def PS(name, shape, dt=F32):
        ps[name] = ctx.enter_context(nc.psum_tensor(name, shape, dt))
        return ps[name]

    pe1a = PS("pe1a", [P, NFEAT])
    pe1b = PS("pe1b", [P, NFEAT])
    pst = PS("pst", [P, NFEAT], BF16)
    psh = PS("psh", [P, NHID])
    pso = PS("pso", [NCLASS, P])
    po2 = PS("po2", [P, NCLASS])
    pe1 = [pe1a, pe1b]

    tables = [xt, h_tA, h_tB, h_tA]

    # ---------------- Pool: gathers -------------------------------------
    @block.gpsimd
    def _(gp: bass.BassGpSimd):
        gp.wait_ge(sem["init"], 16)  # gidx loaded (first init DMA)
        for L in range(4):
            F = FDIMS[L]
            if L >= 1:
                gp.wait_ge(sem["hw"], 16 * BPC * L)
                gp.collective_compute(
                    "AllGather", mybir.AluOpType.bypass,
                    replica_groups=[list(range(CORES))],
                    ins=[h_shard[SPLIT * P:BPC * P, :].opt()],
                    outs=[tables[L][CORES * SPLIT * P:NPACK, :].opt()],
                ).then_inc(sem["cc"], 1)
                gp.wait_ge(sem["cc"], 2 * L - 1)
            for t in range(T):
                if L >= 1 and t == W:
                    gp.wait_ge(sem["cc"], 2 * L)
                if L < 3 and t == 676:
                    gp.wait_ge(sem["hw"], 16 * (BPC * L + SPLIT))
                    gp.collective_compute(
                        "AllGather", mybir.AluOpType.bypass,
                        replica_groups=[list(range(CORES))],
                        ins=[h_shard[0:SPLIT * P, :].opt()],
                        outs=[tables[L + 1][0:CORES * SPLIT * P, :].opt()],
                    ).then_inc(sem["cc"], 1)
                if t >= NBUF and t % 8 == 0:
                    gp.wait_ge(sem[f"mm{L}"], t - NBUF + 8)
                gp.indirect_dma_start(
                    out=gbuf[:, t % NBUF, :F],
                    out_offset=None,
                    in_=tables[L][:, :],
                    in_offset=bass.IndirectOffsetOnAxis(ap=idxs[:, t:t + 1], axis=0),
                ).then_inc(sem[f"g{L}"], 16)

    # ---------------- PE ------------------------------------------------
    @block.tensor
    def _(pe: bass.BassTensorEngine):
        pe.wait_ge(sem["init"], 16 * NINIT)
        pe.wait_ge(sem["dveinit"], 1)

        def tail(L, b):
            if b < 0:
                return
            if L == 0:
                pe.wait_ge(sem["u10"], b + 1)   # hpre(b) ready (DVE)
                pe.wait_ge(sem["u40"], b)       # ACT done reading pst of b-1
                pe.transpose(out=pst[:, 0:P], in_=hpre[:, 0:P], identity=idb[:, :])
                pe.transpose(out=pst[:, P:2 * P], in_=hpre[:, P:2 * P], identity=idb[:, :]).then_inc(sem["v10"], 1)
                pe.wait_ge(sem["u20"], b + 1)   # hpT copied (ACT)
                pe.matmul(out=psh[:, :], lhsT=w1s[:, 0:NHID], rhs=hpT[:, 0:NHID],
                          start=True, stop=False, skip_group_check=True)
                pe.matmul(out=psh[:, :], lhsT=w1s[:, NHID:2 * NHID], rhs=hpT[:, NHID:2 * NHID],
                          start=False, stop=True, skip_group_check=True).then_inc(sem["v20"], 1)
                pe.wait_ge(sem["u30"], b + 1)   # hT relu'd (ACT)
                pe.transpose(out=pst[:, 0:P], in_=hTs[:, :], identity=idb[:, :]).then_inc(sem["v30"], 1)
            elif L == 3:
                pe.wait_ge(sem["u13"], b + 1)   # pre4(b) ready (DVE)
                pe.wait_ge(sem["u23"], b)       # ACT done reading pst of b-1
                pe.transpose(out=pst[:, 0:P], in_=hpre[:, 0:P], identity=idb[:, :]).then_inc(sem["v13"], 1)
                pe.wait_ge(sem["u23"], b + 1)   # p4T copied (ACT)
                pe.matmul(out=pso[:, :], lhsT=w2s[:, :], rhs=p4T[:, :],
                          start=True, stop=True, skip_group_check=True).then_inc(sem["v23"], 1)
                pe.wait_ge(sem["u33"], b + 1)   # oTs relu'd (ACT)
                pe.wait_ge(sem["a23"], b)       # ACT exp of b-1 done reading po2
                pe.wait_ge(sem["m23"], b)       # DVE final of b-1 done reading po2
                pe.transpose(out=po2[:, :], in_=oTs[:, :], identity=idf[0:NCLASS, 0:NCLASS]).then_inc(sem["v33"], 1)

        for L in range(4):
            F = FDIMS[L]
            for b in range(BPC):
                if b >= 2:
                    pe.wait_ge(sem[f"t{L}"], b - 1)
                elif L >= 1:
                    pe.wait_ge(sem[f"t{L-1}"], BPC)
                pe.matmul(out=pe1[b % 2][:, :F], lhsT=zer[:, 0:P], rhs=zer[:, :F],
                          start=True, stop=False, skip_group_check=True)
                for k in range(tpb):
                    t = b * tpb + k
                    # +4-instruction slack: gsem is a 16-wide per-DMA counter
                    # whose engines can complete out of order; the slack (capped
                    # at the layer total, which implies full completion) bounds
                    # the skew under which tile t's data is guaranteed landed.
                    pe.wait_ge(sem[f"g{L}"], 16 * min(t + 17, T))
                    pe.wait_ge(sem[f"s{L}"], t + 1)
                    pe.matmul(out=pe1[b % 2][:, :F], lhsT=sval[:, t % NSV, :],
                              rhs=gbuf[:, t % NBUF, :F], start=False,
                              stop=(k == tpb - 1),
                              skip_group_check=True).then_inc(sem[f"mm{L}"], 1)
                tail(L, b - 1)
            tail(L, BPC - 1)

    # ---------------- DVE -----------------------------------------------
    @block.vector
    def _(dv: bass.BassVectorEngine):
        dv.memset(zer[:, :], 0)
        dv.sem_inc(sem["dveinit"], 1)
        dv.wait_ge(sem["init"], 16 * NINIT)

        def tail(L, b):
            if b < 0:
                return
            dv.wait_ge(sem[f"mm{L}"], (b + 1) * tpb)
            F = FDIMS[L]
            pp = pe1[b % 2]
            if L == 0:
                dv.tensor_tensor(out=tmp[:, :F], in0=pp[:, :F], in1=d1s[:, :F],
                                 op=mybir.AluOpType.mult).then_inc(sem["t0"], 1)
                dv.wait_ge(sem["v10"], b)       # PE done transposing hpre(b-1)
                dv.tensor_tensor(out=hpre[:, :F], in0=xow[:, b, :], in1=tmp[:, :F],
                                 op=mybir.AluOpType.subtract).then_inc(sem["u10"], 1)
            elif L in (1, 2):
                dv.tensor_tensor(out=tmp[:, :F], in0=pp[:, :F], in1=hds[:, (L - 1) * NHID:L * NHID],
                                 op=mybir.AluOpType.mult).then_inc(sem[f"t{L}"], 1)
                dv.wait_ge(sem[f"u2{L}"], b)    # ACT done reading hpre(b-1)
                dv.tensor_tensor(out=hpre[:, 0:F], in0=how[:, b, :], in1=tmp[:, :F],
                                 op=mybir.AluOpType.subtract).then_inc(sem[f"u1{L}"], 1)
            else:
                dv.tensor_tensor(out=tmp[:, :F], in0=pp[:, :F], in1=d2s[:, :F],
                                 op=mybir.AluOpType.mult).then_inc(sem["t3"], 1)
                dv.wait_ge(sem["v13"], b)       # PE done transposing hpre(b-1)
                dv.tensor_tensor(out=hpre[:, 0:F], in0=how[:, b, :], in1=tmp[:, :F],
                                 op=mybir.AluOpType.subtract).then_inc(sem["u13"], 1)
                dv.wait_ge(sem["v33"], b + 1)
                dv.tensor_reduce(out=nmx[:, :], in_=po2[:, :], axis=mybir.AxisListType.X,
                                 op=mybir.AluOpType.max, negate=True).then_inc(sem["m13"], 1)
                dv.wait_ge(sem["a23"], b + 1)
                if b >= 2:
                    dv.wait_ge(sem["os"], 16 * (b - 1))  # osb slot free
                dv.tensor_scalar(out=osb[:, b % 2, :], in0=po2[:, :], scalar1=nmx[:, :1],
                                 scalar2=lse[:, :1], op0=mybir.AluOpType.add,
                                 op1=mybir.AluOpType.subtract).then_inc(sem["m23"], 1)

        for L in range(4):
            if L >= 1:
                dv.wait_ge(sem[f"mm{L-1}"], T)
            for b in range(BPC):
                for k in range(tpb):
                    t = b * tpb + k
                    if t >= NSV and t % 8 == 0:
                        dv.wait_ge(sem[f"mm{L}"], t - NSV + 8)
                    dv.tensor_scalar(out=sval[:, t % NSV, :], in0=ios[:, :],
                                     scalar1=dls[:, t:t + 1], scalar2=vls[:, t:t + 1],
                                     op0=mybir.AluOpType.is_equal,
                                     op1=mybir.AluOpType.mult).then_inc(sem[f"s{L}"], 1)
                tail(L, b - 1)
            tail(L, BPC - 1)

    # ---------------- ACT -----------------------------------------------
    @block.scalar
    def _(ac: bass.BassScalarEngine):
        AF = mybir.ActivationFunctionType
        for src, dst in [(gidx, idxs), (gdl, dls), (gvl, vls), (b1p, b1s),
                         (b2p, b2s), (d1p, d1s), (d2p, d2s), (iop, ios),
                         (idbf, idb), (idf32, idf), (w2p, w2s)]:
            ac.dma_start(out=dst[:, :], in_=src[:, :]).then_inc(sem["init"], 16)
        ac.dma_start(out=w1s[:, :], in_=w1p[:, :]).then_inc(sem["init"], 16)
        ac.dma_start(out=hds[:, :], in_=hdp[:, :]).then_inc(sem["init"], 16)
        for b in range(BPC):
            ac.dma_start(out=xow[:, b, :], in_=xloc[b * P:(b + 1) * P, :]).then_inc(sem["init"], 16)

        def tail(L, b):
            if b < 0:
                return
            if L == 0:
                ac.wait_ge(sem["v10"], b + 1)
                ac.activation(out=hpT[:, :], in_=pst[:, :],
                              func=AF.Copy).then_inc(sem["u20"], 1)
                ac.wait_ge(sem["v20"], b + 1)
                ac.activation(out=hTs[:, :], in_=psh[:, :], func=AF.Relu,
                              bias=b1s[:, :1]).then_inc(sem["u30"], 1)
                ac.wait_ge(sem["v30"], b + 1)
                ac.activation(out=how[:, b, :], in_=pst[:, 0:P], func=AF.Copy).then_inc(sem["u40"], 1)
                ac.dma_start(out=h_shard[b * P:(b + 1) * P, :], in_=how[:, b, :]).then_inc(sem["hw"], 16)
            elif L in (1, 2):
                ac.wait_ge(sem[f"u1{L}"], b + 1)
                ac.activation(out=how[:, b, :], in_=hpre[:, 0:NHID],
                              func=AF.Relu).then_inc(sem[f"u2{L}"], 1)
                ac.dma_start(out=h_shard[b * P:(b + 1) * P, :], in_=how[:, b, :]).then_inc(sem["hw"], 16)
            else:
                ac.wait_ge(sem["v13"], b + 1)
                ac.activation(out=p4T[:, :], in_=pst[:, 0:P], func=AF.Copy).then_inc(sem["u23"], 1)
                ac.wait_ge(sem["v23"], b + 1)
                ac.activation(out=oTs[:, :], in_=pso[:, :], func=AF.Relu,
                              bias=b2s[:, :1]).then_inc(sem["u33"], 1)
                ac.wait_ge(sem["m13"], b + 1)
                ac.activation(out=esb[:, :], in_=po2[:, :], func=AF.Exp,
                              bias=nmx[:, :1], accum_out=sxp[:, :1])
                ac.activation(out=lse[:, :], in_=sxp[:, :], func=AF.Ln).then_inc(sem["a23"], 1)
                ac.wait_ge(sem["m23"], b + 1)
                ac.dma_start(out=outp[b * P:(b + 1) * P, :], in_=osb[:, b % 2, :]).then_inc(sem["os"], 16)

        for L in range(4):
            if L in (1, 2):
                ac.wait_ge(sem["cc"], 2 * L)
            for b in range(BPC):
                tail(L, b - 1)
            tail(L, BPC - 1)
        ac.wait_ge(sem["os"], 16 * BPC)

    ctx.close()
    nc.compile()
    return nc


_CACHE = {}


def kernel(x, rows, cols, vals, diag1, W1, b1, hidden_diags, diag2, W2, b2):
    x = np.asarray(x)
    rows = np.asarray(rows).astype(np.int64)
    cols = np.asarray(cols).astype(np.int64)
    vals = np.asarray(vals)
    pos, tpos, tpb, W, gidx, gdl, gvl = _pack_graph(rows, cols, vals)

    if (tpb, W) not in _CACHE:
        _CACHE[(tpb, W)] = _build(tpb, W)
    nc = _CACHE[(tpb, W)]

    x_packed = np.zeros((NPACK, NFEAT), BF)
    x_packed[tpos] = np.asarray(x).astype(BF)
    x_shard = np.zeros((NPACK, NFEAT), BF)
    x_shard[pos] = np.asarray(x).astype(BF)
    iota = np.tile(np.arange(P, dtype=np.float32)[None, :], (P, 1))
    ident = np.eye(P, dtype=np.float32)
    d1 = np.tile((np.asarray(diag1) + 1.0).astype(np.float32)[None, :], (P, 1))
    hd = np.tile(np.asarray(hidden_diags).astype(np.float32).reshape(1, -1), (P, 1))
    d2 = np.tile((np.asarray(diag2) + 1.0).astype(np.float32)[None, :], (P, 1))
    w1 = np.asarray(W1).astype(BF).reshape(2, P, NHID).transpose(1, 0, 2).reshape(P, 2 * NHID)
    w2 = np.asarray(W2).astype(BF)
    b1c = np.asarray(b1).astype(np.float32)[:, None]
    b2c = np.asarray(b2).astype(np.float32)[:, None]

    in_maps = []
    for c in range(CORES):
        in_maps.append({
            "xt": x_packed,
            "xloc": x_shard[c * RPC:(c + 1) * RPC],
            "gidx": gidx[c], "gdl": gdl[c], "gvl": gvl[c],
            "w1p": w1, "w2p": w2, "b1p": b1c, "b2p": b2c,
            "d1p": d1, "hdp": hd, "d2p": d2,
            "iop": iota, "idbf": ident.astype(BF), "idf32": ident,
        })

    res = run_bass_kernel_spmd(nc, in_maps, core_ids=list(range(CORES)))
    out_packed = np.concatenate([res.results[c]["out"] for c in range(CORES)], axis=0)
    return out_packed[pos].astype(np.float32)



# revision 8
# speedup vs baseline: 1.1315x; 1.1315x over previous
"""AdaGNN on 8 TRN2 NeuronCores (Bass, SPMD) — v2.

vs v1 (per-tile indirect DMA gathers):
- Algebraic fold: h1 = x@W1 + b1 + L@(x@(-M1)) with M1 = diag(diag1+1)@W1,
  hidden: h' = relu(h + L@(h*(-hd))), out = h3@W2 + b2 + L@(h3@(-M2)).
  Every layer's spmm gathers from a 128-wide bf16 z-table; the diag
  multiplies, per-block subtracts, and most PE transposes disappear.
- Gathers use batched dma_gather (SWDGE ucode, ~1us fixed cost per
  instruction amortized over a whole block's tiles): 2 instructions per
  dest block (z-tables split lo/hi at 32768 rows for int16 indices)
  instead of one indirect DMA per 128-slot tile.
- 3 AllGather chunks per layer boundary (A: blocks 0..31 -> lo table,
  C2: 32..42, C3: 43..48 -> hi table) overlap collectives with compute.
"""

import math
import numpy as np
import ml_dtypes

import concourse.bacc as bacc
import concourse.bass as bass
import concourse.mybir as mybir
from concourse.bass_utils import run_bass_kernel_spmd
from concourse.library_config import mlp

N = 50000
E = 800000
NFEAT = 256
NHID = 128
NCLASS = 40
CORES = 8
P = 128
BPC = 49                 # blocks per core
NBLK = CORES * BPC       # 392
RPC = BPC * P            # 6272 rows per core
NA_B = 32                # region-A blocks per core (lo table)
NC2_B = 11               # region-C2 blocks
NC3_B = 6                # region-C3 blocks
LO_ROWS = CORES * NA_B * P    # 32768 (int16-addressable)
HI_ROWS = CORES * (BPC - NA_B) * P  # 17408
C2_ROWS = CORES * NC2_B * P   # 11264 (hi-local)
TPBMAX = 22
GR = 4                   # gbuf ring depth (blocks)
SR = 3                   # sval ring depth (blocks)

F32 = mybir.dt.float32
BF16 = mybir.dt.bfloat16
I16 = mybir.dt.int16
BF = ml_dtypes.bfloat16


# ----------------------------------------------------------------------------
# Host-side graph packing
# ----------------------------------------------------------------------------

def _pack_graph(rows, cols, vals):
    """Load-balance nodes into 392 blocks of 128 by in-degree; split each
    block's edges by source table half (lo = region A < 32768); build per-core
    wrapped int16 gather indices + dest-local/val tile arrays."""
    indeg = np.bincount(rows, minlength=N).astype(np.int64)
    order = np.argsort(-indeg, kind="stable")
    import heapq
    heap = [(0, b) for b in range(NBLK)]
    heapq.heapify(heap)
    bcount = np.zeros(NBLK, np.int64)
    bsum = np.zeros(NBLK, np.int64)
    pos = np.empty(N, np.int64)
    for v in order:
        while True:
            s, b = heapq.heappop(heap)
            if bcount[b] < P:
                break
        pos[v] = b * P + bcount[b]
        bcount[b] += 1
        bsum[b] += indeg[v]
        if bcount[b] < P:
            heapq.heappush(heap, (bsum[b], b))

    blk_all = pos // P
    loc_all = pos % P
    c_all = blk_all // BPC
    b_all = blk_all % BPC
    tpos = np.where(
        b_all < NA_B,
        c_all * (NA_B * P) + b_all * P + loc_all,
        np.where(
            b_all < NA_B + NC2_B,
            LO_ROWS + c_all * (NC2_B * P) + (b_all - NA_B) * P + loc_all,
            LO_ROWS + C2_ROWS + c_all * (NC3_B * P) + (b_all - NA_B - NC2_B) * P + loc_all,
        ),
    )

    d_pos = pos[rows]
    s_pos = tpos[cols]
    blk = d_pos // P
    eorder = np.argsort(blk * 2 + (s_pos >= LO_ROWS), kind="stable")
    blk_s = blk[eorder]
    dloc_s = (d_pos % P)[eorder]
    src_s = s_pos[eorder]
    val_s = np.asarray(vals)[eorder].astype(np.float32)
    islo_s = src_s < LO_ROWS
    starts = np.searchsorted(blk_s, np.arange(NBLK + 1))

    # per-(core, block) lo/hi edge counts -> shared tile schedule (max over cores)
    nlo = np.zeros((CORES, BPC), np.int64)
    nhi = np.zeros((CORES, BPC), np.int64)
    for b in range(NBLK):
        lo_cnt = int(islo_s[starts[b]:starts[b + 1]].sum())
        c, bc = b // BPC, b % BPC
        nlo[c, bc] = lo_cnt
        nhi[c, bc] = (starts[b + 1] - starts[b]) - lo_cnt
    tl = np.maximum(1, -(-nlo.max(axis=0) // P))   # tiles for lo half, per block
    th = np.maximum(1, -(-nhi.max(axis=0) // P))   # tiles for hi half
    tiles = tl + th
    assert tiles.max() <= TPBMAX, f"block needs {tiles.max()} tiles > {TPBMAX}"
    tb = np.zeros(BPC + 1, np.int64)
    tb[1:] = np.cumsum(tiles)
    T1 = int(tb[-1])

    gidx = np.zeros((CORES, P, T1), np.int16)
    gdl = np.full((CORES, P, T1), -1.0, np.float32)
    gvl = np.zeros((CORES, P, T1), np.float32)
    for b in range(NBLK):
        lo, hi = starts[b], starts[b + 1]
        c, bc = b // BPC, b % BPC
        n_lo = int(nlo[c, bc])
        base = tb[bc]
        for half, (e0, e1, toff) in enumerate(
            [(lo, lo + n_lo, base), (lo + n_lo, hi, base + tl[bc])]
        ):
            n = e1 - e0
            if n == 0:
                continue
            sl = np.arange(n)
            tt = toff + sl // P
            pp = sl % P
            s = src_s[e0:e1]
            gidx[c, pp, tt] = (s - (LO_ROWS if half else 0)).astype(np.int16)
            gdl[c, pp, tt] = dloc_s[e0:e1]
            gvl[c, pp, tt] = val_s[e0:e1]

    # wrapped int16 index layout: per tile t, flat j = k*128+p ->
    # idxw[j%16 (+16r replicas), 8*t + j//16]
    idxw = np.zeros((CORES, 16, 8 * T1), np.int16)
    j = np.arange(P)
    for t in range(T1):
        # tile-local flat index j (0..127) -> row j%16, col 8*t + j//16
        idxw[:, j % 16, 8 * t + j // 16] = gidx[:, j, t]
    idxw = np.tile(idxw, (1, 8, 1))
    return pos, tpos, tl, th, tb, T1, idxw, gdl, gvl


# ----------------------------------------------------------------------------
# Bass graph
# ----------------------------------------------------------------------------

def _build(tl, th, tb, T1):
    tl = [int(x) for x in tl]
    th = [int(x) for x in th]
    tb = [int(x) for x in tb]
    # dma_gather payload cap: 8 tiles (1024 idxs -> 16KB per DMA engine)
    MAXT = 8
    seg_lo = [-(-t // MAXT) for t in tl]
    seg_hi = [-(-t // MAXT) for t in th]
    # per-lane cumulative gather counts (lane = b % 4)
    def cum(segs):
        c, tot = [0] * BPC, [0] * 4
        for b in range(BPC):
            tot[b % 4] += segs[b]
            c[b] = tot[b % 4]
        return c, tot
    clo, SL = cum(seg_lo)
    chi, SH = cum(seg_hi)
    NINIT = 14

    nc = bacc.Bacc("TRN2")

    xTp = nc.declare_dram_parameter("xTp", [P, 2 * RPC], BF16, isOutput=False)
    idxp = nc.declare_dram_parameter("idxp", [P, 8 * T1], I16, isOutput=False)
    dlp = nc.declare_dram_parameter("dlp", [P, T1], F32, isOutput=False)
    vlp = nc.declare_dram_parameter("vlp", [P, T1], F32, isOutput=False)
    w1p = nc.declare_dram_parameter("w1p", [P, 2 * NHID], BF16, isOutput=False)
    m1p = nc.declare_dram_parameter("m1p", [P, 2 * NHID], BF16, isOutput=False)
    w2p = nc.declare_dram_parameter("w2p", [P, NCLASS], BF16, isOutput=False)
    m2p = nc.declare_dram_parameter("m2p", [P, NCLASS], BF16, isOutput=False)
    hd1p = nc.declare_dram_parameter("hd1p", [P, NHID], BF16, isOutput=False)
    hd2p = nc.declare_dram_parameter("hd2p", [P, NHID], BF16, isOutput=False)
    browp = nc.declare_dram_parameter("browp", [P, 2 * NHID], BF16, isOutput=False)
    onesp = nc.declare_dram_parameter("onesp", [P, NHID], BF16, isOutput=False)
    iop = nc.declare_dram_parameter("iop", [P, P], BF16, isOutput=False)
    idbp = nc.declare_dram_parameter("idbp", [P, P], BF16, isOutput=False)
    outp = nc.declare_dram_parameter("out", [RPC, NCLASS], F32, isOutput=True)

    tlo = [nc.dram_tensor(f"tlo{i}", [LO_ROWS, NHID], BF16) for i in range(4)]
    thi = [nc.dram_tensor(f"thi{i}", [HI_ROWS, NHID], BF16) for i in range(4)]
    zsh = [nc.dram_tensor(f"zsh{i}", [RPC, NHID], BF16) for i in range(4)]

    import contextlib
    ctx = contextlib.ExitStack()
    block = ctx.enter_context(nc.Block())
    sem = {}
    for nm in (["init", "dvi", "cc", "zw", "vP", "cP", "mm", "s", "u", "zv",
                "tq", "hq", "zc", "zq", "m1", "m2", "a2", "os"]
               + [f"g{i}" for i in range(4)] + [f"h{i}" for i in range(4)]):
        sem[nm] = ctx.enter_context(nc.semaphore(nm))

    sb = {}
    def S(name, shape, dt):
        sb[name] = ctx.enter_context(nc.sbuf_tensor(name, shape, dt))
        return sb[name]

    gbuf = S("gbuf", [P, GR, TPBMAX, NHID], BF16)
    sval = S("sval", [P, SR, TPBMAX, P], BF16)
    idxs = S("idxs", [P, 8 * T1], I16)
    dls = S("dls", [P, T1], F32)
    vls = S("vls", [P, T1], F32)
    xTs = S("xTs", [P, 2 * RPC], BF16)
    how = S("how", [P, BPC, NHID], BF16)
    h3T = S("h3T", [P, BPC, P], BF16)
    w1s = S("w1s", [P, 2 * NHID], BF16)
    m1s = S("m1s", [P, 2 * NHID], BF16)
    w2s = S("w2s", [P, NCLASS], BF16)
    m2s = S("m2s", [P, NCLASS], BF16)
    hd1s = S("hd1s", [P, NHID], BF16)
    hd2s = S("hd2s", [P, NHID], BF16)
    brows = S("brows", [P, 2 * NHID], BF16)
    oness = S("oness", [P, NHID], BF16)
    ioss = S("ioss", [P, P], BF16)
    idbs = S("idbs", [P, P], BF16)
    zsb = S("zsb", [P, 2, NHID], BF16)
    zsb3 = S("zsb3", [P, 2, NHID], BF16)
    ob = S("ob", [P, 2, NCLASS], F32)
    esb = S("esb", [P, NCLASS], F32)
    osb = S("osb", [P, 2, NCLASS], F32)
    nmx = S("nmx", [P, 1], F32)
    sxp = S("sxp", [P, 1], F32)
    lse = S("lse", [P, 1], F32)

    ps = {}
    def PS(name, shape, dt=F32):
        ps[name] = ctx.enter_context(nc.psum_tensor(name, shape, dt))
        return ps[name]

    pe1 = [PS("pe1a", [P, NHID]), PS("pe1b", [P, NHID])]
    po = [PS("poa", [P, NCLASS]), PS("pob", [P, NCLASS])]
    pz = PS("pz", [P, NCLASS])
    pst = PS("pst", [P, P], BF16)

    AG = [("AllGather", mybir.AluOpType.bypass)]

    def fire(gp, i, which):
        """Fire AllGather chunk `which` of boundary i: zsh[i] -> tables[i]."""
        if which == 0:
            ins, outs = zsh[i][0:NA_B * P, :], tlo[i][:, :]
        elif which == 1:
            ins = zsh[i][NA_B * P:(NA_B + NC2_B) * P, :]
            outs = thi[i][0:C2_ROWS, :]
        else:
            ins = zsh[i][(NA_B + NC2_B) * P:RPC, :]
            outs = thi[i][C2_ROWS:HI_ROWS, :]
        gp.collective_compute(
            "AllGather", mybir.AluOpType.bypass,
            replica_groups=[list(range(CORES))],
            ins=[ins.opt()], outs=[outs.opt()],
        ).then_inc(sem["cc"], 1)

    # ---------------- gpsimd: batched gathers + collectives ----------------
    @block.gpsimd
    def _(gp: bass.BassGpSimd):
        gp.load_library(mlp)
        gp.wait_ge(sem["init"], 16 * NINIT)
        # boundary 0 collectives (z0 written by preamble)
        gp.wait_ge(sem["zw"], 16 * NA_B)
        fire(gp, 0, 0)
        gp.wait_ge(sem["zw"], 16 * (NA_B + NC2_B))
        fire(gp, 0, 1)
        gp.wait_ge(sem["zw"], 16 * BPC)
        fire(gp, 0, 2)
        for L in range(4):
            if L >= 1:
                gp.wait_ge(sem["mm"], T1 * L)   # ring slots of prev layer free
            for b in range(BPC):
                if b == 0:
                    gp.wait_ge(sem["cc"], 3 * L + 1)   # lo table (A) ready
                if b >= GR:
                    gp.wait_ge(sem["mm"], T1 * L + tb[b - GR + 1])
                c0 = 8 * tb[b]
                for j in range(seg_lo[b]):
                    k0, k1 = j * MAXT, min((j + 1) * MAXT, tl[b])
                    gp.dma_gather(
                        gbuf[:, b % GR, k0:k1, :],
                        tlo[L][:, :],
                        idxs[:, c0 + 8 * k0:c0 + 8 * k1],
                        (k1 - k0) * P, (k1 - k0) * P, NHID,
                    ).then_inc(sem[f"g{b % 4}"], 16)
                if b == 0:
                    gp.wait_ge(sem["cc"], 3 * L + 3)   # hi table (C2+C3) ready
                for j in range(seg_hi[b]):
                    k0 = tl[b] + j * MAXT
                    k1 = min(tl[b] + (j + 1) * MAXT, tl[b] + th[b])
                    gp.dma_gather(
                        gbuf[:, b % GR, k0:k1, :],
                        thi[L][:, :],
                        idxs[:, c0 + 8 * k0:c0 + 8 * k1],
                        (k1 - k0) * P, (k1 - k0) * P, NHID,
                    ).then_inc(sem[f"h{b % 4}"], 16)
                if L < 3 and b == NA_B + 1:
                    gp.wait_ge(sem["zw"], 16 * (BPC * (L + 1) + NA_B))
                    fire(gp, L + 1, 0)
                if L < 3 and b == NA_B + NC2_B + 1:
                    gp.wait_ge(sem["zw"], 16 * (BPC * (L + 1) + NA_B + NC2_B))
                    fire(gp, L + 1, 1)
            if L < 3:
                gp.wait_ge(sem["zw"], 16 * BPC * (L + 2))
                fire(gp, L + 1, 2)

    # ---------------- PE ---------------------------------------------------
    @block.tensor
    def _(pe: bass.BassTensorEngine):
        pe.wait_ge(sem["init"], 16 * NINIT)
        # preamble: z0(b) = x_b @ (-M1)
        for b in range(BPC):
            if b >= 2:
                pe.wait_ge(sem["cP"], b - 1)
            mmi = None
            for j in range(2):
                mmi = pe.matmul(out=pe1[b % 2][:, :],
                                lhsT=xTs[:, j * RPC + b * P:j * RPC + (b + 1) * P],
                                rhs=m1s[:, j * NHID:(j + 1) * NHID],
                                start=(j == 0), stop=(j == 1),
                                skip_group_check=True)
            mmi.then_inc(sem["vP"], 1)

        def tail2(b):
            # L2 tail: transpose h3(b), z3(b) = h3(b) @ (-M2)
            if b < 0:
                return
            pe.wait_ge(sem["u"], 2 * BPC + b + 1)       # how(b)=h3(b) written
            if b >= 1:
                pe.wait_ge(sem["hq"], b)                 # pst free
            pe.transpose(out=pst[:, :], in_=how[:, b, :],
                         identity=idbs[:, :]).then_inc(sem["tq"], 1)
            pe.wait_ge(sem["hq"], b + 1)                 # h3T(b) copied
            if b >= 1:
                pe.wait_ge(sem["zc"], b)                 # pz free
            pe.matmul(out=pz[:, :], lhsT=h3T[:, b, :], rhs=m2s[:, :],
                      start=True, stop=True,
                      skip_group_check=True).then_inc(sem["zq"], 1)

        for L in range(4):
            for b in range(BPC):
                bank = po[b % 2] if L == 3 else pe1[b % 2]
                F = NCLASS if L == 3 else NHID
                # bank free: reader is ACT relu (sem u, cumulative), except L0
                # where preamble's ACT copy (cP) freed it
                if L == 0:
                    pe.wait_ge(sem["cP"], BPC if b < 2 else 0)
                    if b >= 2:
                        pe.wait_ge(sem["u"], b - 1)
                else:
                    pe.wait_ge(sem["u"], BPC * L + b - 1 if b >= 2 else BPC * L)
                # seed
                if L == 0:
                    for j in range(2):
                        pe.matmul(out=bank[:, :],
                                  lhsT=xTs[:, j * RPC + b * P:j * RPC + (b + 1) * P],
                                  rhs=w1s[:, j * NHID:(j + 1) * NHID],
                                  start=(j == 0), stop=False, skip_group_check=True)
                    pe.matmul(out=bank[:, :], lhsT=oness[0:1, :],
                              rhs=brows[0:1, 0:NHID],
                              start=False, stop=False, skip_group_check=True)
                elif L in (1, 2):
                    pe.matmul(out=bank[:, :], lhsT=idbs[:, :], rhs=how[:, b, :],
                              start=True, stop=False, skip_group_check=True)
                else:
                    pe.wait_ge(sem["hq"], b + 1)
                    pe.matmul(out=bank[:, :], lhsT=h3T[:, b, :], rhs=w2s[:, :],
                              start=True, stop=False, skip_group_check=True)
                    pe.matmul(out=bank[:, :], lhsT=oness[0:1, :],
                              rhs=brows[0:1, NHID:NHID + NCLASS],
                              start=False, stop=False, skip_group_check=True)
                # spmm tiles
                lane = b % 4
                pe.wait_ge(sem[f"g{lane}"], 16 * (L * SL[lane] + clo[b]))
                for k in range(tl[b] + th[b]):
                    t = tb[b] + k
                    if k == tl[b]:
                        pe.wait_ge(sem[f"h{lane}"], 16 * (L * SH[lane] + chi[b]))
                    pe.wait_ge(sem["s"], T1 * L + t + 1)
                    pe.matmul(out=bank[:, :F], lhsT=sval[:, b % SR, k, :],
                              rhs=gbuf[:, b % GR, k, 0:F], start=False,
                              stop=(k == tl[b] + th[b] - 1),
                              skip_group_check=True).then_inc(sem["mm"], 1)
                if L == 2:
                    tail2(b - 1)
            if L == 2:
                tail2(BPC - 1)

    # ---------------- DVE ---------------------------------------------------
    @block.vector
    def _(dv: bass.BassVectorEngine):
        dv.memset(zsb3[:, 0, :], 0)
        dv.memset(zsb3[:, 1, :], 0)
        dv.sem_inc(sem["dvi"], 1)
        dv.wait_ge(sem["init"], 16 * NINIT)

        def tail(L, b):
            if b < 0:
                return
            if L in (0, 1):
                # z_{L+1}(b) = h(b) * (-hd)
                dv.wait_ge(sem["u"], BPC * L + b + 1)        # how(b) written
                dv.wait_ge(sem["zw"], 16 * (BPC * (L + 1) + b - 1)
                           if b >= 2 else 16 * BPC * (L + 1))  # zsb slot free
                dv.tensor_tensor(out=zsb[:, b % 2, :], in0=how[:, b, :],
                                 in1=hd1s[:, :] if L == 0 else hd2s[:, :],
                                 op=mybir.AluOpType.mult).then_inc(sem["zv"], 1)
            elif L == 3:
                dv.wait_ge(sem["u"], 3 * BPC + b + 1)        # ob(b) ready
                dv.wait_ge(sem["a2"], b)                     # nmx free
                dv.tensor_reduce(out=nmx[:, :], in_=ob[:, b % 2, :],
                                 axis=mybir.AxisListType.X,
                                 op=mybir.AluOpType.max,
                                 negate=True).then_inc(sem["m1"], 1)
                dv.wait_ge(sem["a2"], b + 1)                 # lse ready
                if b >= 2:
                    dv.wait_ge(sem["os"], 16 * (b - 1))      # osb slot free
                dv.tensor_scalar(out=osb[:, b % 2, :], in0=ob[:, b % 2, :],
                                 scalar1=nmx[:, :1], scalar2=lse[:, :1],
                                 op0=mybir.AluOpType.add,
                                 op1=mybir.AluOpType.subtract).then_inc(sem["m2"], 1)

        for L in range(4):
            if L >= 1:
                dv.wait_ge(sem["mm"], T1 * L)   # sval ring of prev layer free
            for b in range(BPC):
                if b >= SR:
                    dv.wait_ge(sem["mm"], T1 * L + tb[b - SR + 1])
                for k in range(tl[b] + th[b]):
                    t = tb[b] + k
                    dv.tensor_scalar(out=sval[:, b % SR, k, :], in0=ioss[:, :],
                                     scalar1=dls[:, t:t + 1],
                                     scalar2=vls[:, t:t + 1],
                                     op0=mybir.AluOpType.is_equal,
                                     op1=mybir.AluOpType.mult).then_inc(sem["s"], 1)
                tail(L, b - 1)
            tail(L, BPC - 1)

    # ---------------- ACT ---------------------------------------------------
    @block.scalar
    def _(ac: bass.BassScalarEngine):
        AF = mybir.ActivationFunctionType
        for src, dst in [(idxp, idxs), (dlp, dls), (vlp, vls), (xTp, xTs),
                         (w1p, w1s), (m1p, m1s), (w2p, w2s), (m2p, m2s),
                         (hd1p, hd1s), (hd2p, hd2s), (browp, brows),
                         (onesp, oness), (iop, ioss), (idbp, idbs)]:
            ac.dma_start(out=dst[:, :], in_=src[:, :]).then_inc(sem["init"], 16)

        # preamble: copy z0 psum -> zsb -> zsh[0]
        for b in range(BPC):
            ac.wait_ge(sem["vP"], b + 1)
            if b >= 2:
                ac.wait_ge(sem["zw"], 16 * (b - 1))
            ac.activation(out=zsb[:, b % 2, :], in_=pe1[b % 2][:, :],
                          func=AF.Copy).then_inc(sem["cP"], 1)
            ac.dma_start(out=zsh[0][b * P:(b + 1) * P, :],
                         in_=zsb[:, b % 2, :]).then_inc(sem["zw"], 16)

        def tail(L, b):
            if b < 0:
                return
            if L in (0, 1):
                ac.wait_ge(sem["mm"], T1 * L + tb[b + 1])
                ac.activation(out=how[:, b, :], in_=pe1[b % 2][:, :],
                              func=AF.Relu).then_inc(sem["u"], 1)
                ac.wait_ge(sem["zv"], BPC * L + b + 1)
                ac.dma_start(out=zsh[L + 1][b * P:(b + 1) * P, :],
                             in_=zsb[:, b % 2, :]).then_inc(sem["zw"], 16)
            elif L == 2:
                ac.wait_ge(sem["mm"], T1 * 2 + tb[b + 1])
                ac.activation(out=how[:, b, :], in_=pe1[b % 2][:, :],
                              func=AF.Relu).then_inc(sem["u"], 1)
                ac.wait_ge(sem["tq"], b + 1)
                ac.activation(out=h3T[:, b, :], in_=pst[:, :],
                              func=AF.Copy).then_inc(sem["hq"], 1)
                ac.wait_ge(sem["zq"], b + 1)
                if b >= 2:
                    ac.wait_ge(sem["zw"], 16 * (3 * BPC + b - 1))
                if b == 0:
                    ac.wait_ge(sem["dvi"], 1)
                ac.activation(out=zsb3[:, b % 2, 0:NCLASS], in_=pz[:, :],
                              func=AF.Copy).then_inc(sem["zc"], 1)
                ac.dma_start(out=zsh[3][b * P:(b + 1) * P, :],
                             in_=zsb3[:, b % 2, :]).then_inc(sem["zw"], 16)
            else:
                ac.wait_ge(sem["mm"], T1 * 3 + tb[b + 1])
                ac.activation(out=ob[:, b % 2, :], in_=po[b % 2][:, :],
                              func=AF.Relu).then_inc(sem["u"], 1)
                ac.wait_ge(sem["m1"], b + 1)
                ac.wait_ge(sem["m2"], b)                     # lse free
                ac.activation(out=esb[:, :], in_=ob[:, b % 2, :], func=AF.Exp,
                              bias=nmx[:, :1], accum_out=sxp[:, :1])
                ac.activation(out=lse[:, :], in_=sxp[:, :],
                              func=AF.Ln).then_inc(sem["a2"], 1)
                ac.wait_ge(sem["m2"], b + 1)
                ac.dma_start(out=outp[b * P:(b + 1) * P, :],
                             in_=osb[:, b % 2, :]).then_inc(sem["os"], 16)

        for L in range(4):
            for b in range(BPC):
                tail(L, b - 1)
            tail(L, BPC - 1)
        ac.wait_ge(sem["os"], 16 * BPC)

    ctx.close()
    nc.compile()
    return nc


_CACHE = {}


def kernel(x, rows, cols, vals, diag1, W1, b1, hidden_diags, diag2, W2, b2):
    x = np.asarray(x)
    rows = np.asarray(rows).astype(np.int64)
    cols = np.asarray(cols).astype(np.int64)
    vals = np.asarray(vals)
    pos, tpos, tl, th, tb, T1, idxw, gdl, gvl = _pack_graph(rows, cols, vals)

    key = (tuple(int(v) for v in tl), tuple(int(v) for v in th))
    if key not in _CACHE:
        _CACHE[key] = _build(tl, th, tb, T1)
    nc = _CACHE[key]

    # per-core transposed x in block-slot order: xT[p, j*RPC + n] = xloc[n, j*128+p]
    x_shard = np.zeros((CORES * RPC, NFEAT), np.float32)
    x_shard[pos] = x.astype(np.float32)
    d1 = (np.asarray(diag1).astype(np.float32) + 1.0)
    d2 = (np.asarray(diag2).astype(np.float32) + 1.0)
    W1f = np.asarray(W1).astype(np.float32)
    W2f = np.asarray(W2).astype(np.float32)
    M1n = -(d1[:, None] * W1f)          # (256, 128)
    M2n = -(d2[:, None] * W2f)          # (128, 40)

    def packW(w):  # (256,128) -> [P, 2*NHID] with slab j at cols j*NHID
        return np.ascontiguousarray(
            w.reshape(2, P, NHID).transpose(1, 0, 2).reshape(P, 2 * NHID)
        ).astype(BF)

    w1 = packW(W1f)
    m1 = packW(M1n)
    w2 = W2f.astype(BF)
    m2 = M2n.astype(BF)
    hd = np.asarray(hidden_diags).astype(np.float32)
    hd1 = np.tile((-hd[0])[None, :], (P, 1)).astype(BF)
    hd2 = np.tile((-hd[1])[None, :], (P, 1)).astype(BF)
    brow = np.zeros((P, 2 * NHID), np.float32)
    brow[0, 0:NHID] = np.asarray(b1).astype(np.float32)
    brow[0, NHID:NHID + NCLASS] = np.asarray(b2).astype(np.float32)
    ones = np.ones((P, NHID), np.float32).astype(BF)
    iota = np.tile(np.arange(P, dtype=np.float32)[None, :], (P, 1)).astype(BF)
    ident = np.eye(P, dtype=np.float32).astype(BF)

    in_maps = []
    for c in range(CORES):
        xloc = x_shard[c * RPC:(c + 1) * RPC]               # (6272, 256)
        xT = np.ascontiguousarray(
            xloc.T.reshape(2, P, RPC).transpose(1, 0, 2).reshape(P, 2 * RPC)
        ).astype(BF)
        in_maps.append({
            "xTp": xT, "idxp": idxw[c],
            "dlp": gdl[c], "vlp": gvl[c],
            "w1p": w1, "m1p": m1, "w2p": w2, "m2p": m2,
            "hd1p": hd1, "hd2p": hd2, "browp": brow.astype(BF),
            "onesp": ones, "iop": iota, "idbp": ident,
        })

    res = run_bass_kernel_spmd(nc, in_maps, core_ids=list(range(CORES)))
    out_packed = np.concatenate([res.results[c]["out"] for c in range(CORES)], axis=0)
    return out_packed[pos].astype(np.float32)
